# revision 1
# baseline (speedup 1.0000x reference)
"""ModePool2d (K=3, S=2, P=1, 17 bins) Trainium2 Bass kernel.

Input  x: (8, 64, 224, 224) f32 in [0,1).
Output  : (8, 64, 112, 112) f32 = argmax-bin/16 of the 17-bin histogram
(bin = round-half-even(16x) in [0,16]) over each 3x3 stride-2 window of
the zero-padded image, first-max tie-break — bit-exact vs the jax
reference.

Sharding: pure data-parallel over batch; core k handles batch k (64
channel-images).  Per-core partition p = 2*c + s, s in {0,1} = top /
bottom half of the padded rows, so all 128 partitions are used.  The
host pads/halves the input and reassembles the output.

Algorithm (all exact fp32/fp16 integer arithmetic):
 * qb = 2^23 + round_half_even(16 x) via the fp32 magic-number trick
   (one tensor_scalar; matches jnp.round bit-exactly, including
   half-way ties).
 * Bins processed in pairs (b0, b1 = b0+1) with radix-64 packing.
   Custom DVE ops evaluate, per element of a 113-wide logical grid,
   pack2(r) = eq(r, 2^23+b0) + 64 * eq(r, 2^23+b1).  The three window
   column sets (stride-2 offsets 0/1/2 of qb) are 1-free-dim views, so
   the horizontal 3-sum of pack2 takes TWO custom instructions per bin
   pair (pair-pack of cols 0&1, then accumulate col 2).
 * Vertical 3-sum: one fp16 tensor_tensor add (2x packed mode) plus a
   fused custom op that adds the third row, extracts the two counts
   (round-to-64 magic), forms scores = count + (17-bin)/64 and maxes.
 * Scores max-reduced over the 9 pair groups (fp16 2x), then one fused
   custom op decodes m = c* + (17-b*)/64 into b*/16 (exact).
Ties: equal counts give the smaller bin via the (17-b)/64 bias = the
reference's first-argmax semantics; count differences (>=1) dominate
all biases (<=17/64).
"""

import numpy as np

import concourse.bass as bass
import concourse.mybir as mybir
import concourse.tile as tile
from concourse import bacc
from concourse.bass_utils import run_bass_kernel_spmd

# --------------------------------------------------------------------------
# Custom DVE ops (registered into concourse.dve_ops at import time)
# --------------------------------------------------------------------------
from concourse.dve_spec import (
    Spec, Src0, Src1, C0, C1, C2, maxx, eq, lower,
)
from concourse.dve_ops import (
    DveOp, OPS, CUSTOM_DVE_SPECS, _SUB_OPCODE_FOR_NAME, has_src1,
)
from concourse.dve_uop import DveOpSpec

MAGIC = float(2 ** 23)
K29 = float(2 ** 29)


def _pack2(r, t0, t1):
    r = np.asarray(r, dtype=np.float64)
    return ((r == t0) + 64.0 * (r == t1)).astype(np.float32)


def _ref_h1pair(in0, in1, s0, s1, imm2):
    return (_pack2(in0, s0, s1) + _pack2(in1, s0, s1)).astype(np.float32)


def _ref_p2acc(in0, in1, s0, s1, imm2):
    return (_pack2(in0, s0, s1) + np.asarray(in1, np.float32)).astype(np.float32)


def _ref_dec2v(in0, in1, s0, s1, imm2):
    h2 = np.asarray(in0, np.float64).reshape(in0.shape[0], -1)
    v1 = np.asarray(in1, np.float64).reshape(in1.shape[0], -1)
    v = h2 + v1
    a = np.round(v / 64) * 64  # n0 <= 9 -> round == floor
    return np.maximum((v - a) + s1, a * imm2 + (s1 - imm2)).astype(np.float32)


def _ref_findec(in0, in1, s0, s1, imm2):
    m = np.asarray(in0, np.float64)
    return ((m - np.round(m)) * s1 + imm2).astype(np.float32)


def _ref_findec2(in0, in1, s0, s1, imm2):
    m = np.maximum(np.asarray(in0, np.float64), np.asarray(in1, np.float64))
    return ((m - np.round(m)) * s1 + imm2).astype(np.float32)


MP_H1PAIR_BODY = (eq(Src0, C0) + eq(Src1, C0)) + \
                 (eq(Src0, C1) + eq(Src1, C1)) * C2
MP_P2ACC_BODY = eq(Src0, C0) + eq(Src0, C1) * C2 + Src1
_vv = Src0 + Src1
_av = (_vv + C0) - C0
MP_DEC2V_BODY = maxx((_vv - _av) + C1, _av * C2 + (C1 - C2))
_rr = (Src0 + C0) - C0
MP_FINDEC_BODY = (Src0 - _rr) * C1 + C2
_mm = maxx(Src0, Src1)
_rr2 = (_mm + C0) - C0
MP_FINDEC2_BODY = (_mm - _rr2) * C1 + C2


def _make_op(name, body, reference):
    existing = {op.name: op for op in OPS}
    if name in existing:           # idempotent across re-imports
        return existing[name]
    spec = Spec(body=body, reference=reference)
    opcode = max(_SUB_OPCODE_FOR_NAME.values()) + 1
    shas = {}
    for ver in ("v3", "v4"):
        uops = lower(spec, ver=ver)
        tmp = DveOpSpec(name=name, opcode=opcode, uops=uops,
                        rd1_en=has_src1(spec))
        shas[ver] = tmp.sha(ver)
    op = DveOp(name, spec, subdim=False, uops_sha=shas)
    OPS.append(op)
    CUSTOM_DVE_SPECS[name] = spec
    _SUB_OPCODE_FOR_NAME[name] = opcode
    return op


MP_H1PAIR = _make_op("MP_H1PAIR", MP_H1PAIR_BODY, _ref_h1pair)
MP_P2ACC = _make_op("MP_P2ACC", MP_P2ACC_BODY, _ref_p2acc)
MP_DEC2V = _make_op("MP_DEC2V", MP_DEC2V_BODY, _ref_dec2v)
MP_FINDEC = _make_op("MP_FINDEC", MP_FINDEC_BODY, _ref_findec)
MP_FINDEC2 = _make_op("MP_FINDEC2", MP_FINDEC2_BODY, _ref_findec2)

# --------------------------------------------------------------------------
# Kernel
# --------------------------------------------------------------------------
F32 = mybir.dt.float32
F16 = mybir.dt.float16

B, C, H, W = 8, 64, 224, 224
NCORES = 8
HO, WO = 112, 112
HHALF = 113          # padded rows per half-image
WP = 226             # padded width
ROWS_PER_HALF = 56
CHUNK_OUT = 14       # output rows per chunk
RIN = 2 * CHUNK_OUT + 1
NCHUNK = ROWS_PER_HALF // CHUNK_OUT
VO = CHUNK_OUT * WO
P = 128
G = RIN * 113        # 113-wide logical h grid
HS = 114             # padded h row stride (fp16 4B alignment)
GH = RIN * HS

_CACHE: dict = {}


def _build_program(repeat: int = 1, body_reps: int = 1,
                   qb_on_act: bool = True, v1_on_pool: bool = True,
                   tree_on_pool: bool = True) -> bass.Bass:
    nc = bacc.Bacc("TRN2", target_bir_lowering=False, debug=False)
    x_d = nc.dram_tensor("xin", [P, HHALF, WP], F32, kind="ExternalInput")
    o_d = nc.dram_tensor("out", [P, ROWS_PER_HALF, WO], F32,
                         kind="ExternalOutput")
    AT = mybir.AluOpType
    AF = mybir.ActivationFunctionType

    with tile.TileContext(nc) as tc:
        from contextlib import ExitStack
        with ExitStack() as ctx:
            xpool = ctx.enter_context(tc.tile_pool(name="x", bufs=2))
            qpool = ctx.enter_context(tc.tile_pool(name="q", bufs=2))
            tpool = ctx.enter_context(tc.tile_pool(name="t", bufs=2))
            hpool = ctx.enter_context(tc.tile_pool(name="h", bufs=2))
            vpool = ctx.enter_context(tc.tile_pool(name="v", bufs=2))
            mpool = ctx.enter_context(tc.tile_pool(name="m", bufs=1))
            opool = ctx.enter_context(tc.tile_pool(name="o", bufs=2))

            from contextlib import nullcontext
            bias_ap = None
            if qb_on_act:
                cpool = ctx.enter_context(tc.tile_pool(name="c", bufs=1))
                bias_ap = cpool.tile([P, 1], F32)
                nc.vector.memset(bias_ap[:, :], MAGIC)
            rep_ctx = tc.For_i(0, repeat) if repeat > 1 else nullcontext()
            with rep_ctx:
              for _br in range(body_reps):
                for ch in range(NCHUNK):
                    r0 = 2 * CHUNK_OUT * ch
                    X = xpool.tile([P, RIN * WP + 1], F32)
                    nc.sync.dma_start(
                        X[:, 0:RIN * WP],
                        bass.AP(x_d, r0 * WP, [[HHALF * WP, P], [1, RIN * WP]]))
                    nc.vector.memset(X[:, RIN * WP:], 0.0)
                    qb = qpool.tile([P, RIN * WP + 1], F32)
                    if qb_on_act:
                        nc.scalar.activation(qb[:, :], X[:, :], AF.Identity,
                                             bias=bias_ap[:, :], scale=16.0)
                    else:
                        nc.vector.tensor_scalar(qb[:, :], X[:, :], 16.0, MAGIC,
                                                AT.mult, AT.add)

                    def qview(off, n):
                        return bass.AP(qb.tensor, off,
                                       [[RIN * WP + 1, P], [2, n]])

                    m_all = mpool.tile([P, 9 * VO], F16)

                    def hrows(h, start):
                        return bass.AP(h.tensor, start * HS,
                                       [[GH, P], [2 * HS, CHUNK_OUT],
                                        [1, WO]])

                    def emit_dec2v(g, h, v1):
                        mg = m_all[:, g * VO:(g + 1) * VO]
                        nc.vector._custom_dve(
                            MP_DEC2V, out=mg, in0=hrows(h, 2), in1=v1[:, :],
                            s0=K29, s1=(17.0 - 2 * g) / 64.0, imm2=1.0 / 64.0)

                    # DEC2V lags the pack by one pair so the Pool-engine v1
                    # add overlaps the next pair's eq-pack on the DVE.
                    pend = []
                    for g in range(9):
                        b0, b1 = 2 * g, 2 * g + 1
                        t = tpool.tile([P, G], F16, tag="t")
                        nc.vector._custom_dve(
                            MP_H1PAIR, out=t[:, :],
                            in0=qview(0, G), in1=qview(1, G),
                            s0=MAGIC + b0, s1=MAGIC + b1, imm2=64.0)
                        h = hpool.tile([P, GH], F16, tag="h")
                        h2d = bass.AP(h.tensor, 0,
                                      [[GH, P], [HS, RIN], [1, 113]])
                        nc.vector._custom_dve(
                            MP_P2ACC, out=h2d,
                            in0=qview(2, G), in1=t[:, :],
                            s0=MAGIC + b0, s1=MAGIC + b1, imm2=64.0)
                        v1 = vpool.tile([P, VO], F16, tag="v1")
                        eng = nc.gpsimd if v1_on_pool else nc.vector
                        eng.tensor_tensor(v1[:, :], hrows(h, 0), hrows(h, 1),
                                          AT.add)
                        pend.append((g, h, v1))
                        if len(pend) >= 2:
                            emit_dec2v(*pend.pop(0))
                    while pend:
                        emit_dec2v(*pend.pop(0))

                    # wide max-tree: {0,2,4,6} <- max(.,{1,3,5,7});
                    # {0,4} <- max(.,{2,6}); 0 <- max(0,4); findec2(0, 8)
                    def mview(off, ng, step):
                        return bass.AP(m_all.tensor, off * VO,
                                       [[9 * VO, P], [step * VO, ng], [1, VO]])
                    nc.vector.tensor_tensor(mview(0, 4, 2), mview(0, 4, 2),
                                            mview(1, 4, 2), AT.max)
                    nc.vector.tensor_tensor(mview(0, 2, 4), mview(0, 2, 4),
                                            mview(2, 2, 4), AT.max)
                    nc.vector.tensor_tensor(mview(0, 1, 1), mview(0, 1, 1),
                                            mview(4, 1, 1), AT.max)

                    ot = opool.tile([P, VO], F32, tag="ot")
                    nc.vector._custom_dve(
                        MP_FINDEC2, out=ot[:, :],
                        in0=m_all[:, 0:VO], in1=m_all[:, 8 * VO:9 * VO],
                        s0=MAGIC, s1=-4.0, imm2=17.0 / 16.0)
                    nc.sync.dma_start(
                        bass.AP(o_d, ch * CHUNK_OUT * WO,
                                [[ROWS_PER_HALF * WO, P], [1, VO]]),
                        ot[:, :])
    nc.compile()
    return nc


def _host_prep(x: np.ndarray) -> np.ndarray:
    xp = np.zeros((B, C, 2, HHALF, WP), dtype=np.float32)
    xp[:, :, 0, 1:113, 1:225] = x[:, :, 0:112, :]
    xp[:, :, 1, 0:113, 1:225] = x[:, :, 111:224, :]
    return xp.reshape(B, P, HHALF, WP)


def kernel(x: np.ndarray) -> np.ndarray:
    x = np.asarray(x, dtype=np.float32)
    assert x.shape == (B, C, H, W)
    if "nc" not in _CACHE:
        _CACHE["nc"] = _build_program()
    nc = _CACHE["nc"]
    xp = _host_prep(x)
    in_maps = [{"xin": np.ascontiguousarray(xp[k])} for k in range(NCORES)]
    res = run_bass_kernel_spmd(nc, in_maps, core_ids=list(range(NCORES)))
    out = np.empty((B, C, HO, WO), dtype=np.float32)
    for k in range(NCORES):
        out[k] = res.results[k]["out"].reshape(C, HO, WO)
    return out



# revision 2
# speedup vs baseline: 1.5498x; 1.5498x over previous
"""ModePool2d (K=3, S=2, P=1, 17 bins) Trainium2 Bass kernel.

Input  x: (8, 64, 224, 224) f32 in [0,1).
Output  : (8, 64, 112, 112) f32 = argmax-bin/16 of the 17-bin histogram
(bin = round-half-even(16x) in [0,16]) over each 3x3 stride-2 window of
the zero-padded image, first-max tie-break — bit-exact vs the jax
reference.

Sharding: pure data-parallel over batch; core k handles batch k (64
channel-images).  Per-core partition p = 2*c + s, s in {0,1} = top /
bottom half of the padded rows, so all 128 partitions are used.  The
host pads/halves the input and reassembles the output.

Algorithm (all exact fp32/fp16 integer arithmetic):
 * qb = 2^23 + round_half_even(16 x) via the fp32 magic-number trick
   (one tensor_scalar; matches jnp.round bit-exactly, including
   half-way ties).
 * Bins processed in pairs (b0, b1 = b0+1) with radix-64 packing.
   Custom DVE ops evaluate, per element of a 113-wide logical grid,
   pack2(r) = eq(r, 2^23+b0) + 64 * eq(r, 2^23+b1).  The three window
   column sets (stride-2 offsets 0/1/2 of qb) are 1-free-dim views, so
   the horizontal 3-sum of pack2 takes TWO custom instructions per bin
   pair (pair-pack of cols 0&1, then accumulate col 2).
 * Vertical 3-sum: one fp16 tensor_tensor add (2x packed mode) plus a
   fused custom op that adds the third row, extracts the two counts
   (round-to-64 magic), forms scores = count + (17-bin)/64 and maxes.
 * Scores max-reduced over the 9 pair groups (fp16 2x), then one fused
   custom op decodes m = c* + (17-b*)/64 into b*/16 (exact).
Ties: equal counts give the smaller bin via the (17-b)/64 bias = the
reference's first-argmax semantics; count differences (>=1) dominate
all biases (<=17/64).
"""

import numpy as np

import concourse.bass as bass
import concourse.mybir as mybir
import concourse.tile as tile
from concourse import bacc
from concourse.bass_utils import run_bass_kernel_spmd

# --------------------------------------------------------------------------
# Custom DVE ops (registered into concourse.dve_ops at import time)
# --------------------------------------------------------------------------
from concourse.dve_spec import (
    Spec, Src0, Src1, C0, C1, C2, maxx, eq, lower,
)
from concourse.dve_ops import (
    DveOp, OPS, CUSTOM_DVE_SPECS, _SUB_OPCODE_FOR_NAME, has_src1,
)
from concourse.dve_uop import DveOpSpec

MAGIC = float(2 ** 23)
K29 = float(2 ** 29)


def _pack2(r, t0, t1):
    r = np.asarray(r, dtype=np.float64)
    return ((r == t0) + 64.0 * (r == t1)).astype(np.float32)


def _ref_h1pair(in0, in1, s0, s1, imm2):
    return (_pack2(in0, s0, s1) + _pack2(in1, s0, s1)).astype(np.float32)


def _ref_p2acc(in0, in1, s0, s1, imm2):
    return (_pack2(in0, s0, s1) + np.asarray(in1, np.float32)).astype(np.float32)


def _ref_dec2v(in0, in1, s0, s1, imm2):
    h2 = np.asarray(in0, np.float64).reshape(in0.shape[0], -1)
    v1 = np.asarray(in1, np.float64).reshape(in1.shape[0], -1)
    v = h2 + v1
    a = np.round(v / 64) * 64  # n0 <= 9 -> round == floor
    return np.maximum((v - a) + s1, a * imm2 + (s1 - imm2)).astype(np.float32)


def _ref_findec(in0, in1, s0, s1, imm2):
    m = np.asarray(in0, np.float64)
    return ((m - np.round(m)) * s1 + imm2).astype(np.float32)


def _ref_findec2(in0, in1, s0, s1, imm2):
    m = np.maximum(np.asarray(in0, np.float64), np.asarray(in1, np.float64))
    return ((m - np.round(m)) * s1 + imm2).astype(np.float32)


MP_H1PAIR_BODY = (eq(Src0, C0) + eq(Src1, C0)) + \
                 (eq(Src0, C1) + eq(Src1, C1)) * C2
MP_P2ACC_BODY = eq(Src0, C0) + eq(Src0, C1) * C2 + Src1
_vv = Src0 + Src1
_av = (_vv + C0) - C0
MP_DEC2V_BODY = maxx((_vv - _av) + C1, _av * C2 + (C1 - C2))
_rr = (Src0 + C0) - C0
MP_FINDEC_BODY = (Src0 - _rr) * C1 + C2
_mm = maxx(Src0, Src1)
_rr2 = (_mm + C0) - C0
MP_FINDEC2_BODY = (_mm - _rr2) * C1 + C2


def _make_op(name, body, reference):
    existing = {op.name: op for op in OPS}
    if name in existing:           # idempotent across re-imports
        return existing[name]
    spec = Spec(body=body, reference=reference)
    opcode = max(_SUB_OPCODE_FOR_NAME.values()) + 1
    shas = {}
    for ver in ("v3", "v4"):
        uops = lower(spec, ver=ver)
        tmp = DveOpSpec(name=name, opcode=opcode, uops=uops,
                        rd1_en=has_src1(spec))
        shas[ver] = tmp.sha(ver)
    op = DveOp(name, spec, subdim=False, uops_sha=shas)
    OPS.append(op)
    CUSTOM_DVE_SPECS[name] = spec
    _SUB_OPCODE_FOR_NAME[name] = opcode
    return op


MP_H1PAIR = _make_op("MP_H1PAIR", MP_H1PAIR_BODY, _ref_h1pair)
MP_P2ACC = _make_op("MP_P2ACC", MP_P2ACC_BODY, _ref_p2acc)
MP_DEC2V = _make_op("MP_DEC2V", MP_DEC2V_BODY, _ref_dec2v)
MP_FINDEC = _make_op("MP_FINDEC", MP_FINDEC_BODY, _ref_findec)
MP_FINDEC2 = _make_op("MP_FINDEC2", MP_FINDEC2_BODY, _ref_findec2)

# --------------------------------------------------------------------------
# Kernel
# --------------------------------------------------------------------------
F32 = mybir.dt.float32
F16 = mybir.dt.float16

B, C, H, W = 8, 64, 224, 224
NCORES = 8
HO, WO = 112, 112
HHALF = 113          # padded rows per half-image
WP = 226             # padded width
ROWS_PER_HALF = 56
CHUNK_OUT = 14       # output rows per chunk
RIN = 2 * CHUNK_OUT + 1
NCHUNK = ROWS_PER_HALF // CHUNK_OUT
VO = CHUNK_OUT * WO
P = 128
G = RIN * 113        # 113-wide logical h grid
HS = 114             # padded h row stride (fp16 4B alignment)
GH = RIN * HS

_CACHE: dict = {}


def _build_program(repeat: int = 1, body_reps: int = 1,
                   qb_on_act: bool = True, v1_on_pool: bool = False,
                   tree_on_pool: bool = True) -> bass.Bass:
    nc = bacc.Bacc("TRN2", target_bir_lowering=False, debug=False)
    x_d = nc.dram_tensor("xin", [P, HHALF, WP], F32, kind="ExternalInput")
    o_d = nc.dram_tensor("out", [P, ROWS_PER_HALF, WO], F32,
                         kind="ExternalOutput")
    AT = mybir.AluOpType
    AF = mybir.ActivationFunctionType

    with tile.TileContext(nc) as tc:
        from contextlib import ExitStack
        with ExitStack() as ctx:
            xpool = ctx.enter_context(tc.tile_pool(name="x", bufs=2))
            qpool = ctx.enter_context(tc.tile_pool(name="q", bufs=2))
            tpool = ctx.enter_context(tc.tile_pool(name="t", bufs=2))
            hpool = ctx.enter_context(tc.tile_pool(name="h", bufs=2))
            vpool = ctx.enter_context(tc.tile_pool(name="v", bufs=2))
            mpool = ctx.enter_context(tc.tile_pool(name="m", bufs=1))
            opool = ctx.enter_context(tc.tile_pool(name="o", bufs=2))

            from contextlib import nullcontext
            bias_ap = None
            if qb_on_act:
                cpool = ctx.enter_context(tc.tile_pool(name="c", bufs=1))
                bias_ap = cpool.tile([P, 1], F32)
                nc.vector.memset(bias_ap[:, :], MAGIC)
            rep_ctx = tc.For_i(0, repeat) if repeat > 1 else nullcontext()
            with rep_ctx:
              for _br in range(body_reps):
                for ch in range(NCHUNK):
                    r0 = 2 * CHUNK_OUT * ch
                    X = xpool.tile([P, RIN * WP + 1], F32)
                    nc.sync.dma_start(
                        X[:, 0:RIN * WP],
                        bass.AP(x_d, r0 * WP, [[HHALF * WP, P], [1, RIN * WP]]))
                    nc.vector.memset(X[:, RIN * WP:], 0.0)
                    qb = qpool.tile([P, RIN * WP + 1], F32)
                    if qb_on_act:
                        nc.scalar.activation(qb[:, :], X[:, :], AF.Identity,
                                             bias=bias_ap[:, :], scale=16.0)
                    else:
                        nc.vector.tensor_scalar(qb[:, :], X[:, :], 16.0, MAGIC,
                                                AT.mult, AT.add)

                    def qview(off, n):
                        return bass.AP(qb.tensor, off,
                                       [[RIN * WP + 1, P], [2, n]])

                    m_all = mpool.tile([P, 9 * VO], F16)

                    def hrows(h, start):
                        return bass.AP(h.tensor, start * HS,
                                       [[GH, P], [2 * HS, CHUNK_OUT],
                                        [1, WO]])

                    def emit_dec2v(g, h, v1):
                        mg = m_all[:, g * VO:(g + 1) * VO]
                        nc.vector._custom_dve(
                            MP_DEC2V, out=mg, in0=hrows(h, 2), in1=v1[:, :],
                            s0=K29, s1=(17.0 - 2 * g) / 64.0, imm2=1.0 / 64.0)

                    # DEC2V lags the pack by one pair so the Pool-engine v1
                    # add overlaps the next pair's eq-pack on the DVE.
                    pend = []
                    for g in range(9):
                        b0, b1 = 2 * g, 2 * g + 1
                        t = tpool.tile([P, G], F16, tag="t")
                        nc.vector._custom_dve(
                            MP_H1PAIR, out=t[:, :],
                            in0=qview(0, G), in1=qview(1, G),
                            s0=MAGIC + b0, s1=MAGIC + b1, imm2=64.0)
                        h = hpool.tile([P, GH], F16, tag="h")
                        h2d = bass.AP(h.tensor, 0,
                                      [[GH, P], [HS, RIN], [1, 113]])
                        nc.vector._custom_dve(
                            MP_P2ACC, out=h2d,
                            in0=qview(2, G), in1=t[:, :],
                            s0=MAGIC + b0, s1=MAGIC + b1, imm2=64.0)
                        v1 = vpool.tile([P, VO], F16, tag="v1")
                        eng = nc.gpsimd if v1_on_pool else nc.vector
                        eng.tensor_tensor(v1[:, :], hrows(h, 0), hrows(h, 1),
                                          AT.add)
                        pend.append((g, h, v1))
                        if len(pend) >= 2:
                            emit_dec2v(*pend.pop(0))
                    while pend:
                        emit_dec2v(*pend.pop(0))

                    # wide max-tree: {0,2,4,6} <- max(.,{1,3,5,7});
                    # {0,4} <- max(.,{2,6}); 0 <- max(0,4); findec2(0, 8)
                    def mview(off, ng, step):
                        return bass.AP(m_all.tensor, off * VO,
                                       [[9 * VO, P], [step * VO, ng], [1, VO]])
                    nc.vector.tensor_tensor(mview(0, 4, 2), mview(0, 4, 2),
                                            mview(1, 4, 2), AT.max)
                    nc.vector.tensor_tensor(mview(0, 2, 4), mview(0, 2, 4),
                                            mview(2, 2, 4), AT.max)
                    nc.vector.tensor_tensor(mview(0, 1, 1), mview(0, 1, 1),
                                            mview(4, 1, 1), AT.max)

                    ot = opool.tile([P, VO], F32, tag="ot")
                    nc.vector._custom_dve(
                        MP_FINDEC2, out=ot[:, :],
                        in0=m_all[:, 0:VO], in1=m_all[:, 8 * VO:9 * VO],
                        s0=MAGIC, s1=-4.0, imm2=17.0 / 16.0)
                    nc.sync.dma_start(
                        bass.AP(o_d, ch * CHUNK_OUT * WO,
                                [[ROWS_PER_HALF * WO, P], [1, VO]]),
                        ot[:, :])
    nc.compile()
    return nc


def _host_prep(x: np.ndarray) -> np.ndarray:
    xp = np.zeros((B, C, 2, HHALF, WP), dtype=np.float32)
    xp[:, :, 0, 1:113, 1:225] = x[:, :, 0:112, :]
    xp[:, :, 1, 0:113, 1:225] = x[:, :, 111:224, :]
    return xp.reshape(B, P, HHALF, WP)


def kernel(x: np.ndarray) -> np.ndarray:
    x = np.asarray(x, dtype=np.float32)
    assert x.shape == (B, C, H, W)
    if "nc" not in _CACHE:
        _CACHE["nc"] = _build_program()
    nc = _CACHE["nc"]
    xp = _host_prep(x)
    in_maps = [{"xin": np.ascontiguousarray(xp[k])} for k in range(NCORES)]
    res = run_bass_kernel_spmd(nc, in_maps, core_ids=list(range(NCORES)))
    out = np.empty((B, C, HO, WO), dtype=np.float32)
    for k in range(NCORES):
        out[k] = res.results[k]["out"].reshape(C, HO, WO)
    return out



# revision 22
# speedup vs baseline: 1.7751x; 1.1454x over previous
"""ModePool2d (K=3, S=2, P=1, 17 bins) Trainium2 Bass kernel.

Input  x: (8, 64, 224, 224) f32 in [0,1).
Output  : (8, 64, 112, 112) f32 = argmax-bin/16 of the 17-bin histogram
(bin = round-half-even(16x) in [0,16]) over each 3x3 stride-2 window of
the zero-padded image, first-max tie-break — bit-exact vs the jax
reference.

Sharding: pure data-parallel over batch; core k handles batch k (64
channel-images).  Per-core partition p = 2*c + s, s in {0,1} = top /
bottom half of the padded rows, so all 128 partitions are used.  The
host pads/halves the input, de-interleaves padded columns into an
evens block (cols 0,2,..,224) and an odds block (cols 1,3,..,225), and
reassembles the output.  With that layout every h-phase read on-device
is a flat stride-1 run (custom DVE reads run ~18% faster than
stride-2): evens = [0:G], odds = [G:2G], evens+1 = [1:G+1].

Algorithm (all exact fp32/fp16 integer arithmetic):
 * qb = 2^23 + round_half_even(16 x) via the fp32 magic-number trick,
   computed on the ACT engine in-place over the input tile (matches
   jnp.round bit-exactly, including half-way ties).
 * Bins 0..15 in 8 pairs (b0, b1 = b0+1) with radix-64 packing.
   Custom DVE ops evaluate pack2(r) = eq(r, 2^23+b0) + 64*eq(r,
   2^23+b1); the horizontal 3-sum takes TWO custom instructions per
   pair (pair-pack of evens+odds, then accumulate evens+1).
 * Vertical 3-sum: one fp16 tensor_tensor add (2x packed mode) plus a
   fused custom op (DEC2V) that adds the third row, extracts the two
   counts (round-to-64 magic), forms scores = count + (17-bin)/64 and
   maxes.  A running fp16 max over the 8 pair scores replaces the old
   materialize-then-tree reduction (same cost, no chunk-tail barrier,
   44 KB less SBUF).
 * Bin 16 runs on the otherwise-idle ACT engine: SQ = Square(qb -
   (2^23+16)) in {0,1,4,..,256} then E = Relu(1 - SQ) = [bin==16],
   both exact in f16; window sums via cheap f16 2x adds on the DVE;
   m8 = count16 + 1/64 matches the old pair-8 DEC2V output exactly.
 * FINDEC2 fuses max(m_run, m8) with the final decode of
   m = c* + (17-b*)/64 into b*/16 (exact).
Everything per-pair runs on the DVE alone — per-pair cross-engine
handoffs (Pool v1 adds in an earlier version) measurably stall the
pipeline; only chunk-level ACT handoffs (qb, E16) overlap cleanly.
Ties: equal counts give the smaller bin via the (17-b)/64 bias = the
reference's first-argmax semantics; count differences (>=1) dominate
all biases (<=17/64).
"""

import numpy as np

import concourse.bass as bass
import concourse.mybir as mybir
import concourse.tile as tile
from concourse import bacc
from concourse.bass_utils import run_bass_kernel_spmd

# --------------------------------------------------------------------------
# Custom DVE ops (registered into concourse.dve_ops at import time)
# --------------------------------------------------------------------------
from concourse.dve_spec import (
    Spec, Src0, Src1, C0, C1, C2, maxx, eq, lower,
)
from concourse.dve_ops import (
    DveOp, OPS, CUSTOM_DVE_SPECS, _SUB_OPCODE_FOR_NAME, has_src1,
)
from concourse.dve_uop import DveOpSpec

MAGIC = float(2 ** 23)
QMAGIC = 1024.0   # f16 magic: RNE on f16-convert rounds 16x at integer grid
K29 = float(2 ** 29)


def _pack2(r, t0, t1):
    r = np.asarray(r, dtype=np.float64)
    return ((r == t0) + 64.0 * (r == t1)).astype(np.float32)


def _ref_h1pair(in0, in1, s0, s1, imm2):
    return (_pack2(in0, s0, s1) + _pack2(in1, s0, s1)).astype(np.float32)


def _ref_p2acc(in0, in1, s0, s1, imm2):
    return (_pack2(in0, s0, s1) + np.asarray(in1, np.float32)).astype(np.float32)


def _ref_dec2v(in0, in1, s0, s1, imm2):
    h2 = np.asarray(in0, np.float64).reshape(in0.shape[0], -1)
    v1 = np.asarray(in1, np.float64).reshape(in1.shape[0], -1)
    v = h2 + v1
    a = np.round(v / 64) * 64  # n0 <= 9 -> round == floor
    return np.maximum((v - a) + s1, a * imm2 + (s1 - imm2)).astype(np.float32)


def _ref_findec(in0, in1, s0, s1, imm2):
    m = np.asarray(in0, np.float64)
    return ((m - np.round(m)) * s1 + imm2).astype(np.float32)


def _ref_findec2(in0, in1, s0, s1, imm2):
    m = np.maximum(np.asarray(in0, np.float64), np.asarray(in1, np.float64))
    return ((m - np.round(m)) * s1 + imm2).astype(np.float32)


MP_H1PAIR_BODY = (eq(Src0, C0) + eq(Src1, C0)) + \
                 (eq(Src0, C1) + eq(Src1, C1)) * C2
MP_P2ACC_BODY = eq(Src0, C0) + eq(Src0, C1) * C2 + Src1
_vv = Src0 + Src1
_av = (_vv + C0) - C0
MP_DEC2V_BODY = maxx((_vv - _av) + C1, _av * C2 + (C1 - C2))
_rr = (Src0 + C0) - C0
MP_FINDEC_BODY = (Src0 - _rr) * C1 + C2
_mm = maxx(Src0, Src1)
_rr2 = (_mm + C0) - C0
MP_FINDEC2_BODY = (_mm - _rr2) * C1 + C2


def _make_op(name, body, reference):
    existing = {op.name: op for op in OPS}
    if name in existing:           # idempotent across re-imports
        return existing[name]
    spec = Spec(body=body, reference=reference)
    opcode = max(_SUB_OPCODE_FOR_NAME.values()) + 1
    shas = {}
    for ver in ("v3", "v4"):
        uops = lower(spec, ver=ver)
        tmp = DveOpSpec(name=name, opcode=opcode, uops=uops,
                        rd1_en=has_src1(spec))
        shas[ver] = tmp.sha(ver)
    op = DveOp(name, spec, subdim=False, uops_sha=shas)
    OPS.append(op)
    CUSTOM_DVE_SPECS[name] = spec
    _SUB_OPCODE_FOR_NAME[name] = opcode
    return op


MP_H1PAIR = _make_op("MP_H1PAIR", MP_H1PAIR_BODY, _ref_h1pair)
MP_P2ACC = _make_op("MP_P2ACC", MP_P2ACC_BODY, _ref_p2acc)
MP_DEC2V = _make_op("MP_DEC2V", MP_DEC2V_BODY, _ref_dec2v)
MP_FINDEC = _make_op("MP_FINDEC", MP_FINDEC_BODY, _ref_findec)
MP_FINDEC2 = _make_op("MP_FINDEC2", MP_FINDEC2_BODY, _ref_findec2)

# --------------------------------------------------------------------------
# Kernel
# --------------------------------------------------------------------------
F32 = mybir.dt.float32
F16 = mybir.dt.float16

B, C, H, W = 8, 64, 224, 224
NCORES = 8
HO, WO = 112, 112
HHALF = 113          # padded rows per half-image
WP = 226             # padded width
ROWS_PER_HALF = 56
CHUNK_OUT = 14       # output rows per chunk
RIN = 2 * CHUNK_OUT + 1
NCHUNK = ROWS_PER_HALF // CHUNK_OUT
VO = CHUNK_OUT * WO
P = 128
G = RIN * 113        # 113-wide logical h grid
HS = 114             # padded h row stride (fp16 4B alignment)
GH = RIN * HS

_CACHE: dict = {}


def _build_program(repeat: int = 1, body_reps: int = 1,
                   qb_on_act: bool = True, v1_on_pool: bool = False,
                   e_on_pool: bool = False) -> bass.Bass:
    nc = bacc.Bacc("TRN2", target_bir_lowering=False, debug=False)
    # xin layout: [partition, col-parity, padded-row, 113] — the host
    # de-interleaves padded columns into an evens block (cols 0,2,..,224)
    # and an odds block (cols 1,3,..,225) so every h-phase view on-device
    # is a flat stride-1 run.
    x_d = nc.dram_tensor("xin", [P, 2, HHALF, 113], F32, kind="ExternalInput")
    o_d = nc.dram_tensor("out", [P, ROWS_PER_HALF, WO], F32,
                         kind="ExternalOutput")
    AT = mybir.AluOpType
    AF = mybir.ActivationFunctionType

    with tile.TileContext(nc) as tc:
        from contextlib import ExitStack
        with ExitStack() as ctx:
            xpool = ctx.enter_context(tc.tile_pool(name="x", bufs=2))
            qpool = ctx.enter_context(tc.tile_pool(name="q", bufs=2))
            tpool = ctx.enter_context(tc.tile_pool(name="t", bufs=3))
            hpool = ctx.enter_context(tc.tile_pool(name="h", bufs=3))
            vpool = ctx.enter_context(tc.tile_pool(name="v", bufs=3))
            mpool = ctx.enter_context(tc.tile_pool(name="m", bufs=2))
            opool = ctx.enter_context(tc.tile_pool(name="o", bufs=2))

            sqpool = ctx.enter_context(tc.tile_pool(name="sq", bufs=1))
            epool = ctx.enter_context(tc.tile_pool(name="e", bufs=2))
            hepool = ctx.enter_context(tc.tile_pool(name="he", bufs=2))

            from contextlib import nullcontext
            bias_ap = None
            if qb_on_act:
                cpool = ctx.enter_context(tc.tile_pool(name="c", bufs=1))
                bias_ap = cpool.tile([P, 1], F32)
                nc.vector.memset(bias_ap[:, :], MAGIC)
                bias16_ap = cpool.tile([P, 1], F32)
                nc.vector.memset(bias16_ap[:, :], -(MAGIC + 16.0))
                bias1_ap = cpool.tile([P, 1], F32)
                nc.vector.memset(bias1_ap[:, :], 1.0)
            rep_ctx = tc.For_i(0, repeat) if repeat > 1 else nullcontext()
            with rep_ctx:
              for _br in range(body_reps):
                for ch in range(NCHUNK):
                    r0 = 2 * CHUNK_OUT * ch
                    # X/qb layout: [evens RIN*113][odds RIN*113] flat blocks
                    # (host pre-de-interleaved).  All three h-phase views are
                    # stride-1: evens = [0:G], odds = [G:2G], evens+1 =
                    # [1:G+1] (the spill into odds[0] lands in the unused
                    # 113th h column).
                    X = xpool.tile([P, 2 * G], F32)
                    nc.sync.dma_start(
                        X[:, :],
                        bass.AP(x_d, r0 * 113,
                                [[2 * HHALF * 113, P], [HHALF * 113, 2],
                                 [1, G]]))
                    # qb in-place over X (saves 52 KB of SBUF)
                    qb = X
                    if qb_on_act:
                        nc.scalar.activation(qb[:, :], X[:, :], AF.Identity,
                                             bias=bias_ap[:, :], scale=16.0)
                    else:
                        nc.vector.tensor_scalar(qb[:, :], X[:, :], 16.0, MAGIC,
                                                AT.mult, AT.add)

                    def qview(off, n):
                        return bass.AP(qb.tensor, off, [[2 * G, P], [1, n]])

                    # --- bin 16 on the ACT engine (exact Square+Relu
                    # indicator), freeing pair-8's two pack ops on the DVE.
                    # SQ = (bin-16)^2 in {0,1,4,..,256}; E = relu(1-SQ) =
                    # [bin==16], both exact in f16.  h/v sums via cheap f16
                    # 2x adds; m8 = count16 + 1/64 matches the old pair-8
                    # DEC2V output, so FINDEC2 is unchanged.
                    SQ = sqpool.tile([P, 2 * G], F16)
                    nc.scalar.activation(SQ[:, :], qb[:, :], AF.Square,
                                         bias=bias16_ap[:, :], scale=1.0)
                    E16 = epool.tile([P, 2 * G], F16)
                    nc.scalar.activation(E16[:, :], SQ[:, :], AF.Relu,
                                         bias=bias1_ap[:, :], scale=-1.0)

                    def hrows(h, start):
                        return bass.AP(h.tensor, start * HS,
                                       [[GH, P], [2 * HS, CHUNK_OUT],
                                        [1, WO]])

                    # Running max over pair groups: DEC2V(g=0) seeds m_run;
                    # g=1..7 write mg then max-accumulate; g=8 is kept
                    # separate and merged by FINDEC2.  No end-of-chunk tree
                    # barrier; everything pipelines per pair on the DVE.
                    m_run = mpool.tile([P, VO], F16, tag="mrun")
                    m8 = mpool.tile([P, VO], F16, tag="m8")
                    for g in range(8):
                        b0, b1 = 2 * g, 2 * g + 1
                        t = tpool.tile([P, G], F16, tag="t")
                        nc.vector._custom_dve(
                            MP_H1PAIR, out=t[:, :],
                            in0=qview(0, G), in1=qview(G, G),
                            s0=MAGIC + b0, s1=MAGIC + b1, imm2=64.0)
                        h = hpool.tile([P, GH], F16, tag="h")
                        h2d = bass.AP(h.tensor, 0,
                                      [[GH, P], [HS, RIN], [1, 113]])
                        nc.vector._custom_dve(
                            MP_P2ACC, out=h2d,
                            in0=qview(1, G), in1=t[:, :],
                            s0=MAGIC + b0, s1=MAGIC + b1, imm2=64.0)
                        v1 = vpool.tile([P, VO], F16, tag="v1")
                        eng = nc.gpsimd if v1_on_pool else nc.vector
                        eng.tensor_tensor(v1[:, :], hrows(h, 0), hrows(h, 1),
                                          AT.add)
                        if g == 0:
                            dec_out = m_run
                        else:
                            dec_out = vpool.tile([P, VO], F16, tag="mg")
                        nc.vector._custom_dve(
                            MP_DEC2V, out=dec_out[:, :],
                            in0=hrows(h, 2), in1=v1[:, :],
                            s0=K29, s1=(17.0 - 2 * g) / 64.0, imm2=1.0 / 64.0)
                        if g >= 1:
                            nc.vector.tensor_tensor(m_run[:, :], m_run[:, :],
                                                    dec_out[:, :], AT.max)

                    # bin-16 window sums from E16 -> m8, entirely on the Pool
                    # engine (chunk-granularity handoff; only FINDEC2 waits)
                    hE = hepool.tile([P, G], F16)
                    e_eng = nc.gpsimd if e_on_pool else nc.vector
                    e_eng.tensor_tensor(hE[:, :], E16[:, 0:G],
                                        E16[:, G:2 * G], AT.add)
                    e_eng.tensor_tensor(hE[:, :], hE[:, :],
                                        E16[:, 1:G + 1], AT.add)

                    def herows(start):
                        return bass.AP(hE.tensor, start * 113,
                                       [[G, P], [2 * 113, CHUNK_OUT],
                                        [1, WO]])
                    v1E = vpool.tile([P, VO], F16, tag="v1")
                    e_eng.tensor_tensor(v1E[:, :], herows(0), herows(1),
                                        AT.add)
                    nc.vector.scalar_tensor_tensor(
                        m8[:, :], herows(2), 1.0 / 64.0, v1E[:, :],
                        AT.add, AT.add)

                    ot = opool.tile([P, VO], F32, tag="ot")
                    nc.vector._custom_dve(
                        MP_FINDEC2, out=ot[:, :],
                        in0=m_run[:, :], in1=m8[:, :],
                        s0=MAGIC, s1=-4.0, imm2=17.0 / 16.0)
                    nc.sync.dma_start(
                        bass.AP(o_d, ch * CHUNK_OUT * WO,
                                [[ROWS_PER_HALF * WO, P], [1, VO]]),
                        ot[:, :])
    nc.compile()
    return nc


def _host_prep(x: np.ndarray) -> np.ndarray:
    # Padded halves with column parity de-interleaved:
    # evens block = padded cols 0,2,..,224 = [0, x[:,1], x[:,3], .., x[:,223]]
    # odds  block = padded cols 1,3,..,225 = [x[:,0], x[:,2], .., x[:,222], 0]
    xp = np.zeros((B, C, 2, 2, HHALF, 113), dtype=np.float32)
    xp[:, :, 0, 0, 1:113, 1:113] = x[:, :, 0:112, 1:224:2]
    xp[:, :, 0, 1, 1:113, 0:112] = x[:, :, 0:112, 0:224:2]
    xp[:, :, 1, 0, 0:113, 1:113] = x[:, :, 111:224, 1:224:2]
    xp[:, :, 1, 1, 0:113, 0:112] = x[:, :, 111:224, 0:224:2]
    return xp.reshape(B, P, 2, HHALF, 113)


def kernel(x: np.ndarray) -> np.ndarray:
    x = np.asarray(x, dtype=np.float32)
    assert x.shape == (B, C, H, W)
    if "nc" not in _CACHE:
        _CACHE["nc"] = _build_program()
    nc = _CACHE["nc"]
    xp = _host_prep(x)
    in_maps = [{"xin": np.ascontiguousarray(xp[k])} for k in range(NCORES)]
    res = run_bass_kernel_spmd(nc, in_maps, core_ids=list(range(NCORES)))
    out = np.empty((B, C, HO, WO), dtype=np.float32)
    for k in range(NCORES):
        out[k] = res.results[k]["out"].reshape(C, HO, WO)
    return out

